# revision 22
# baseline (speedup 1.0000x reference)
"""Morphological dilation (7x7 additive SE, zero 'same' padding) on 8 trn2 cores.

out[b,c,i,j] = max_{a,t} ( xpad[b,c,i+a,j+t] + w[a,t] ),  x: (8,8,512,512) f32.

Sharding: pure data parallel - 64 images (B*C) split 8 per core; the 7x7
weight is replicated (baked into the program as immediates). No cross-core
communication.

Default variant ("w3"): hand-authored custom DVE uop programs compute a
fused 3-tap sliding max-plus in ONE 1x Vector instruction:

    W3ACC:  out[s] = max(acc[s], x[s]+C2, x[s-1]+C1, x[s-2]+C0)

Per 2-image unit and vertical tap a (slab sh_a = row-shifted padded images,
[128, 8, 518] fp16; partition = row within 128-row strip, free = strip x
padded row):
  - horizontal taps b=0..2 and b=3..5: one W3ACC/W3INIT each (weights as
    instruction immediates via the delta trick C0-C2, C1-C2, C2),
  - tap b=6: ACT add (bias immediate) + one fp16 2x tensor_tensor max.
That is 22 DVE instructions per unit instead of ~66 for the tap-per-
instruction schedule: 49 taps cost 14x(4144+58)cyc + 8x(58+4096/2)cyc
~ 82us per unit on DVE vs ~136us before.

Accumulators are flat [128, 4152] fp16 tiles; each W3 group's output view
is offset by 6-b so its valid window lands at phys col r*518 + 8 + i for
every b. Stream positions whose 3-window crosses a row boundary write
garbage into cols {6-b, 7-b} < 8 of each row -- outside the extracted
region [8, 520) -- so no masking or subdim handling is needed. The first
two outputs of the whole stream are stale-pipeline garbage in the same
discarded columns.
"""

import os
import sys

for p in ("/root/.axon_site", "/root/.axon_site/_ro/trn_rl_repo",
          "/root/.axon_site/_ro/pypackages", "/opt/trn_rl_repo"):
    if os.path.isdir(p) and p not in sys.path:
        sys.path.append(p)

from dataclasses import dataclass

import numpy as np

import concourse.bass as bass  # noqa: F401
import concourse.bacc as bacc
import concourse.mybir as mybir
import concourse.dve_ops as dve_ops
from concourse.bass_utils import run_bass_kernel_spmd
from concourse.dve_spec import Spec, Src0, Src1, C0 as _C0, C1 as _C1, maxx
from concourse.dve_uop import (
    AluInp, AluOp, DelayInp, DveOpSpec, InpSel, OutPath, OutSel, Trigger,
    UopConfig, UopDpConfig, ENABLE,
)
from concourse.tile import TileContext

KH = KW = 7
PAD = 3
H = W = 512
N_CORES = 8
IMGS_PER_CORE = 8  # 8*8 = 64 images total
WPAD = W + 2 * PAD  # 518
S = H // 128  # 4 strips of 128 rows per image
SR = 8  # slab rows per 2-image unit (2 images x 4 strips)
FLAT = SR * WPAD  # 4144
ACCW = 4152  # acc tile width (>= 6 + FLAT, even)

f32 = mybir.dt.float32
f16 = mybir.dt.float16
ADD = mybir.AluOpType.add
MAX = mybir.AluOpType.max
IDENT = mybir.ActivationFunctionType.Identity

VARIANT = os.environ.get("BASS_DILATE_VARIANT", "w3")
N_ACT_EXTRA = int(os.environ.get("BASS_DILATE_ACT_EXTRA", "13"))

# ---- expconv variant constants -------------------------------------------
# Dilation via log-sum-exp: out = G + wmax + (1/beta)*( ln(conv2d(
# e^{beta(x-G)+S}, e^{beta(w-wmax)+T})) - S - T ), computed as banded
# matmuls on the PE. Row-blocks of 128 padded rows at stride 122 (out-rows
# per block M=122); per-(image,block) shift G = max(block max, 0).
# S and T re-center each bf16 factor: the dominant tap of a window can sit
# e^{-beta*D} below its factor's peak with D_x + D_w = G + wmax - out
# (measured <= 6.43 on the real data; D_w <= w spread 5.19). The HW ACT Ln
# table is only valid for inputs in ~[1.2e-38, 3e19], so Ln gets a pre-scale
# e^{-C} to recenter E (its ~126-nat span fits the ~131-nat valid window).
# Validated in numpy (bf16 factors, f32 accum): beta=19 -> worst rel ~0.011.
BETA = float(os.environ.get("BASS_DILATE_BETA", "12.5"))
SSHIFT = float(os.environ.get("BASS_DILATE_S", "42.0"))
TSHIFT = float(os.environ.get("BASS_DILATE_T", "42.0"))
CSHIFT = float(os.environ.get("BASS_DILATE_C", "45.5"))
# The log-sum-exp bias is one-sided (overestimates); subtracting its midpoint
# halves the worst-case error. Measured on the real data at beta=12.5.
OFFSET = float(os.environ.get("BASS_DILATE_OFF", "0.0655"))
R0S = (0, 122, 244, 366, 488)  # padded-row block starts
BKS = (128, 128, 128, 128, 30)  # contraction size (valid padded rows)
BMS = (122, 122, 122, 122, 24)  # out rows per block
NBLK = len(R0S)
bf16 = mybir.dt.bfloat16


# --------------------------------------------------------------------------
# Custom DVE ops: fused 3-tap sliding max-plus (hand-authored v3 uops).
#
# 8-block datapath, 1x mode:
#   b0: a2 = ADD(ch0=x, ch4=c2)           ; ch5 <- CURR(b0)  = a2[s-1]
#   b1: BYPASS(ch5) -> flop a2[s-1]       ; ch0 <- CURR(b1)  = a2[s-2]
#                                         ; ch4 <- PREV_ALU  = a2[s]
#   b2: a1 = ADD(PREV_ALU, ch3=d1)
#   b3: a0 = ADD(ch0, ch2=d0)             ; ch5 <- PREV_ALU  = a1
#   b4: m1 = MAX(PREV_ALU=a0, ch5=a1)
#   b5: m2 = MAX(PREV_ALU, ch4=a2[s])
#   b6: r  = MAX(PREV_ALU, ch1=acc)       [W3INIT: BYPASS]
#   b7: BYPASS -> WR0_LO
# --------------------------------------------------------------------------


def _w3_uop(with_acc: bool) -> UopConfig:
    u = UopConfig()
    u.enable_input(InpSel.SRC_0, 1)      # chain0 = x
    if with_acc:
        u.enable_input(InpSel.SRC_1, 2)  # chain1 = acc
    u.enable_input(InpSel.CONST_0, 3)    # chain2 = d0 = c0-c2
    u.enable_input(InpSel.CONST_1, 4)    # chain3 = d1 = c1-c2
    u.enable_input(InpSel.CONST_2, 5)    # chain4 = c2
    u.require_inp0 = ENABLE
    u.require_inp1 = ENABLE if with_acc else 0
    u.trigger = (Trigger.SRC_TENSOR_DONE, Trigger.NONE, Trigger.NONE)
    u.next_uop = (0, 0, 0)
    u.repeat_count = 0
    u.out[OutPath.WR0_LO] = OutSel.ALU_OUT
    u.out_enable[OutPath.WR0_LO] = ENABLE

    dp = [UopDpConfig() for _ in range(8)]
    dp[0].enable_alu(AluOp.ADD, AluInp.PREV_DELAY_0, AluInp.PREV_DELAY_4)
    dp[0].pass_through_delay(1, 2, 3)
    dp[0].enable_delay_from_src(DelayInp.CURR_ALU_OUT, 5)
    dp[1].enable_alu(AluOp.BYPASS, AluInp.PREV_DELAY_5, AluInp.PREV_DELAY_5)
    dp[1].pass_through_delay(1, 2, 3)
    dp[1].enable_delay_from_src(DelayInp.CURR_ALU_OUT, 0)
    dp[1].enable_delay_from_src(DelayInp.PREV_ALU_OUT, 4)
    dp[2].enable_alu(AluOp.ADD, AluInp.PREV_ALU_OUT, AluInp.PREV_DELAY_3)
    dp[2].pass_through_delay(0, 1, 2, 4)
    dp[3].enable_alu(AluOp.ADD, AluInp.PREV_DELAY_0, AluInp.PREV_DELAY_2)
    dp[3].pass_through_delay(1, 4)
    dp[3].enable_delay_from_src(DelayInp.PREV_ALU_OUT, 5)
    dp[4].enable_alu(AluOp.MAX, AluInp.PREV_ALU_OUT, AluInp.PREV_DELAY_5)
    dp[4].pass_through_delay(1, 4)
    dp[5].enable_alu(AluOp.MAX, AluInp.PREV_ALU_OUT, AluInp.PREV_DELAY_4)
    dp[5].pass_through_delay(1)
    if with_acc:
        dp[6].enable_alu(AluOp.MAX, AluInp.PREV_ALU_OUT, AluInp.PREV_DELAY_1)
    else:
        dp[6].pass_through_alu()
    dp[7].pass_through_alu()
    u.datapath_config = dp
    return u


def _w3_ref_core(in0, s0, s1, imm2):
    p = in0.shape[0]
    x = np.asarray(in0, np.float32).reshape(p, -1)
    a2 = x + float(imm2)
    ninf = np.float32(-np.inf)
    s0 = float(np.asarray(s0).flat[0]) if not isinstance(s0, float) else s0
    s1 = float(np.asarray(s1).flat[0]) if not isinstance(s1, float) else s1
    a1 = np.concatenate([np.full((p, 1), ninf, np.float32), a2[:, :-1] + s1], 1)
    a0 = np.concatenate([np.full((p, 2), ninf, np.float32), a2[:, :-2] + s0], 1)
    return np.maximum(np.maximum(a0, a1), a2)


def _w3acc_ref(in0, in1, s0, s1, imm2):
    r = _w3_ref_core(in0, s0, s1, imm2)
    r = np.maximum(r, np.asarray(in1, np.float32).reshape(r.shape))
    return r.reshape(np.asarray(in0).shape)


def _w3init_ref(in0, in1, s0, s1, imm2):
    return _w3_ref_core(in0, s0, s1, imm2).reshape(np.asarray(in0).shape)


@dataclass(frozen=True)
class _HandOp:
    """Duck-typed DveOp with a hand-authored uop program."""

    name: str
    spec: Spec
    subdim: bool
    uopspec: DveOpSpec

    def compile(self, ver):
        assert ver == "v3", f"W3 ops are v3-only (got {ver})"
        return self.uopspec


_W3_OPS = None


def _register_w3():
    global _W3_OPS
    if _W3_OPS is not None:
        return _W3_OPS
    by_name = {op.name: op for op in dve_ops.OPS}
    if "W3ACC_ANT" in by_name:  # registered by a sibling module/import
        _W3_OPS = (by_name["W3ACC_ANT"], by_name["W3INIT_ANT"])
        return _W3_OPS
    base = dve_ops._CUSTOM_DVE_ROW_BASE
    row_acc = base + len(dve_ops.OPS)
    row_init = row_acc + 1
    assert row_init < 0x20
    w3acc = _HandOp(
        name="W3ACC_ANT",
        spec=Spec(body=maxx(Src0 + _C0, Src1), reference=_w3acc_ref),
        subdim=False,
        uopspec=DveOpSpec(name="W3ACC_ANT", opcode=row_acc,
                          uops=[_w3_uop(True)], rd1_en=True),
    )
    w3init = _HandOp(
        name="W3INIT_ANT",
        spec=Spec(body=Src0 + _C0 + _C1, reference=_w3init_ref),
        subdim=False,
        uopspec=DveOpSpec(name="W3INIT_ANT", opcode=row_init,
                          uops=[_w3_uop(False)], rd1_en=False),
    )
    for op in (w3acc, w3init):
        op.uopspec.validate("v3")
        dve_ops.OPS.append(op)
        dve_ops._SUB_OPCODE_FOR_NAME[op.name] = op.uopspec.opcode
        dve_ops.CUSTOM_DVE_SPECS[op.name] = op.spec
    _W3_OPS = (w3acc, w3init)
    return _W3_OPS


# --------------------------------------------------------------------------
# Kernel builders
# --------------------------------------------------------------------------


def _emit_pad_fill(nc, xpad, zt, g):
    """Zero xpad[g] (interior gets overwritten by the image afterwards)."""
    for r0 in range(0, WPAD, 128):
        r1 = min(WPAD, r0 + 128)
        nc.sync.dma_start(out=xpad[g, r0:r1, :], in_=zt[0:r1 - r0, :])


def _emit_border_fill(nc, xpad, zt, g):
    """Zero only the 3-wide borders of xpad[g]; the interior fill can then
    run concurrently instead of waiting for a full-frame zero pass."""
    nc.sync.dma_start(out=xpad[g, 0:PAD, :], in_=zt[0:PAD, :])
    nc.sync.dma_start(out=xpad[g, PAD + H:WPAD, :], in_=zt[0:PAD, :])
    # column borders, 128 rows at a time
    for r0 in range(0, H, 128):
        nc.sync.dma_start(out=xpad[g, PAD + r0:PAD + r0 + 128, 0:PAD],
                          in_=zt[0:128, 0:PAD])
        nc.sync.dma_start(out=xpad[g, PAD + r0:PAD + r0 + 128, PAD + W:WPAD],
                          in_=zt[0:128, 0:PAD])


def _build_w3(weight):
    w3acc_op, w3init_op = _register_w3()
    w = np.asarray(weight, np.float64)
    nc = bacc.Bacc("TRN2")
    x = nc.dram_tensor("x", (IMGS_PER_CORE, H, W), f32, kind="ExternalInput")
    wt = nc.dram_tensor("weight", (KH, KW), f32, kind="ExternalInput")
    out = nc.dram_tensor("out", (IMGS_PER_CORE, H, W), f32, kind="ExternalOutput")
    NP = IMGS_PER_CORE // 2
    NCH = 2

    with TileContext(nc) as tc:
        with (
            tc.tile_pool(name="const", bufs=1) as cpool,
            tc.tile_pool(name="dram", bufs=1, space="DRAM") as dpool,
            tc.tile_pool(name="sh", bufs=10) as shpool,
            tc.tile_pool(name="tmp", bufs=6) as tpool,
            tc.tile_pool(name="acc", bufs=3) as apool,
        ):
            # W3 weights ride as immediates; the broadcast tile feeds the
            # ACT-path (tap b=6) per-partition biases.
            w_sb = cpool.tile([128, KH * KW], f32)
            nc.sync.dma_start(
                out=w_sb[:, :],
                in_=wt[:, :].rearrange("a b -> (a b)").unsqueeze(0)
                .broadcast_to([128, KH * KW]),
            )
            zt = cpool.tile([128, WPAD], f16)
            nc.vector.memset(zt[:, :], 0.0)

            xpad = dpool.tile([IMGS_PER_CORE, WPAD, WPAD], f16)

            def fill(g):
                _emit_pad_fill(nc, xpad, zt, g)
                nc.gpsimd.dma_start(
                    out=xpad[g, PAD:PAD + H, PAD:PAD + W], in_=x[g, :, :]
                )

            for g in range(min(4, IMGS_PER_CORE)):
                fill(g)

            for u in range(NP):
                g0 = 2 * u
                accs = [
                    apool.tile([128, ACCW], f16, tag=f"acc{c}", name=f"acc{c}_{u}")
                    for c in range(NCH)
                ]
                for c in range(NCH):
                    nc.vector.memset(accs[c][:, 0:6], 0.0)
                acc_used = [False] * NCH

                def acc_valid(c):
                    return (accs[c][:, 8:8 + FLAT]
                            .rearrange("p (s w) -> p s w", w=WPAD)[:, :, 0:W])

                op_idx = 0
                for a in range(KH):
                    sh = shpool.tile([128, SR, WPAD], f16, tag="sh",
                                     name=f"sh_{u}_{a}")
                    for gg in range(2):
                        nc.sync.dma_start(
                            out=sh[:, 4 * gg:4 * gg + 4, :],
                            in_=xpad[g0 + gg, a:a + H, :]
                            .rearrange("(s p) w -> p s w", p=128),
                        )
                    # ACT's tap b=6 add is emitted first so the Scalar engine
                    # starts on the slab immediately; its max joins later.
                    tmp = tpool.tile([128, SR, W], f16, tag="tmp",
                                     name=f"tmp_{u}_{a}")
                    nc.scalar.activation(
                        out=tmp[:, :, :], in_=sh[:, :, 6:6 + W], func=IDENT,
                        bias=w_sb[:, a * KW + 6:a * KW + 7], scale=1.0,
                    )
                    for b in (0, 3):
                        c = op_idx % NCH
                        op_idx += 1
                        o = 6 - b
                        kwargs = dict(
                            out=accs[c][:, o:o + FLAT],
                            in0=sh[:, :, :],
                            s0=float(w[a, b] - w[a, b + 2]),
                            s1=float(w[a, b + 1] - w[a, b + 2]),
                            imm2=float(w[a, b + 2]),
                        )
                        if acc_used[c]:
                            nc.vector._custom_dve(
                                w3acc_op, in1=accs[c][:, o:o + FLAT], **kwargs)
                        else:
                            nc.vector._custom_dve(w3init_op, **kwargs)
                            acc_used[c] = True
                    c = op_idx % NCH
                    op_idx += 1
                    nc.vector.tensor_tensor(
                        out=acc_valid(c), in0=acc_valid(c), in1=tmp[:, :, :],
                        op=MAX,
                    )
                # combine + store per image so the first output DMA overlaps
                # the second image's combine
                for gg in range(2):
                    r0, r1 = 4 * gg, 4 * gg + 4
                    for c in range(1, NCH):
                        nc.vector.tensor_tensor(
                            out=acc_valid(0)[:, r0:r1, :],
                            in0=acc_valid(0)[:, r0:r1, :],
                            in1=acc_valid(c)[:, r0:r1, :],
                            op=MAX,
                        )
                    nc.gpsimd.dma_start(
                        out=out[g0 + gg].rearrange("(s p) w -> p s w", p=128),
                        in_=acc_valid(0)[:, r0:r1, :],
                    )
                for g in (2 * u + 4, 2 * u + 5):
                    if g < IMGS_PER_CORE:
                        fill(g)
    nc.finalize()
    return nc


def _build_f16p():
    """Fallback: tap-per-instruction schedule (DVE tt/ts + ACT adds)."""
    nc = bacc.Bacc("TRN2")
    x = nc.dram_tensor("x", (IMGS_PER_CORE, H, W), f32, kind="ExternalInput")
    wt = nc.dram_tensor("weight", (KH, KW), f32, kind="ExternalInput")
    out = nc.dram_tensor("out", (IMGS_PER_CORE, H, W), f32, kind="ExternalOutput")
    NCH = int(os.environ.get("BASS_DILATE_NCH", "4"))
    NP = IMGS_PER_CORE // 2

    act_taps = {(a, t) for a in range(KH) for t in range(KW) if t % 2 == 1}
    even_taps = [(a, t) for a in range(KH) for t in range(KW) if t % 2 == 0]
    step = max(1, len(even_taps) // max(1, N_ACT_EXTRA))
    for i in range(0, min(N_ACT_EXTRA, len(even_taps))):
        act_taps.add(even_taps[(i * step) % len(even_taps)])

    with TileContext(nc) as tc:
        with (
            tc.tile_pool(name="const", bufs=1) as cpool,
            tc.tile_pool(name="dram", bufs=1, space="DRAM") as dpool,
            tc.tile_pool(name="sh", bufs=10) as shpool,
            tc.tile_pool(name="tmp", bufs=5) as tpool,
            tc.tile_pool(name="acc", bufs=3) as apool,
        ):
            w_sb = cpool.tile([128, KH * KW], f32)
            nc.sync.dma_start(
                out=w_sb[:, :],
                in_=wt[:, :].rearrange("a b -> (a b)").unsqueeze(0)
                .broadcast_to([128, KH * KW]),
            )
            zt = cpool.tile([128, WPAD], f16)
            nc.vector.memset(zt[:, :], 0.0)

            xpad = dpool.tile([IMGS_PER_CORE, WPAD, WPAD], f16)

            def fill(g):
                _emit_pad_fill(nc, xpad, zt, g)
                nc.gpsimd.dma_start(
                    out=xpad[g, PAD:PAD + H, PAD:PAD + W], in_=x[g, :, :]
                )

            for g in range(min(4, IMGS_PER_CORE)):
                fill(g)
            for u in range(NP):
                g0 = 2 * u
                accs = [
                    apool.tile([128, 2, S, W], f16, tag=f"acc{c}", name=f"acc{c}_{u}")
                    for c in range(NCH)
                ]
                acc_used = [False] * NCH
                for a in range(KH):
                    sh = shpool.tile([128, 2, S, WPAD], f16, tag="sh",
                                     name=f"sh_{u}_{a}")
                    for gg in range(2):
                        nc.sync.dma_start(
                            out=sh[:, gg, :, :],
                            in_=xpad[g0 + gg, a:a + H, :]
                            .rearrange("(s p) w -> p s w", p=128),
                        )
                    for t in range(KW):
                        k = a * KW + t
                        c = k % NCH
                        in0 = sh[:, :, :, t:t + W]
                        if not acc_used[c]:
                            dst = accs[c][:, :, :, :]
                        else:
                            tmp = tpool.tile([128, 2, S, W], f16, tag="tmp",
                                             name=f"tmp_{u}_{k}")
                            dst = tmp[:, :, :, :]
                        if (a, t) in act_taps:
                            nc.scalar.activation(
                                out=dst, in_=in0, func=IDENT,
                                bias=w_sb[:, k:k + 1], scale=1.0,
                            )
                        else:
                            nc.vector.tensor_scalar(
                                out=dst, in0=in0,
                                scalar1=w_sb[:, k:k + 1], scalar2=None, op0=ADD,
                            )
                        if acc_used[c]:
                            nc.vector.tensor_tensor(
                                out=accs[c][:, :, :, :], in0=accs[c][:, :, :, :],
                                in1=dst, op=MAX,
                            )
                        acc_used[c] = True
                for c in range(1, NCH):
                    nc.vector.tensor_tensor(
                        out=accs[0][:, :, :, :], in0=accs[0][:, :, :, :],
                        in1=accs[c][:, :, :, :], op=MAX,
                    )
                for gg in range(2):
                    nc.gpsimd.dma_start(
                        out=out[g0 + gg].rearrange("(s p) w -> p s w", p=128),
                        in_=accs[0][:, gg, :, :],
                    )
                for g in (2 * u + 4, 2 * u + 5):
                    if g < IMGS_PER_CORE:
                        fill(g)
    nc.finalize()
    return nc


def _patch_act_tables():
    """Force the Exp+Ln co-resident act-func set (natural_log_exp_and_others)
    so alternating Exp/Ln activations don't reload tables (1.3us per reload,
    ~92us/run). Preserves set order/count so act_func_set_id indices stay
    valid; just strips exp/ln from every other set."""
    import concourse.bacc as _bacc
    import concourse.hw_specs as _hw

    if getattr(_bacc, "_ant_act_patch", False):
        return
    real = _hw.get_activation_tables

    def patched(arch):
        tables = dict(real(arch))
        both = {
            n for n, fs in tables.items()
            if mybir.ActivationFunctionType.Exp in fs
            and mybir.ActivationFunctionType.Ln in fs
        }
        if not both:
            return tables
        keep = next(iter(sorted(both)))
        out = {}
        for n, fs in tables.items():
            if n == keep:
                out[n] = fs
            else:
                out[n] = fs - {mybir.ActivationFunctionType.Exp,
                               mybir.ActivationFunctionType.Ln}
        return out

    _bacc.get_activation_tables = patched
    _bacc._ant_act_patch = True


def _build_expconv():
    """Dilation as exp-domain 7x7 convolution on the tensor engine.

    out[i,j] = max_{a,b} xpad[i+a, j+b] + w[a,b]
            ~= G_p + (1/beta) ln( sum_{a,b} e^{beta w[a,b]} e^{beta(xpad[i+a,j+b]-G_p)} )

    The inner sum is a linear 7x7 conv of Xe = e^{beta(xpad-G_p)} with kernel
    e^{beta w}: for each horizontal tap b, one banded matmul contracting over
    the 128 padded rows of block p (lhsT[r', i'] = e^{beta w[r'-i', b]} band),
    rhs = Xe free-shifted by b, accumulating all 7 taps in PSUM. ACT does
    exp and ln, Pool does the final (1/beta)*ln + G_p, DVE is idle.
    lwT (banded exp-weights) and gb (per-block shifts) are host-computed
    inputs, so the program is weight/data independent.
    """
    _patch_act_tables()
    nc = bacc.Bacc("TRN2")
    x = nc.dram_tensor("x", (IMGS_PER_CORE, H, W), f32, kind="ExternalInput")
    lwT = nc.dram_tensor("lwT", (128, KW, 122), bf16, kind="ExternalInput")
    gb = nc.dram_tensor("gb", (2, IMGS_PER_CORE * NBLK), f32,
                        kind="ExternalInput")
    out = nc.dram_tensor("out", (IMGS_PER_CORE, H, W), f32,
                         kind="ExternalOutput")
    NG = IMGS_PER_CORE * NBLK
    EXPF = mybir.ActivationFunctionType.Exp
    LNF = mybir.ActivationFunctionType.Ln
    MUL = mybir.AluOpType.mult

    with TileContext(nc) as tc:
        with (
            tc.tile_pool(name="const", bufs=1) as cpool,
            tc.tile_pool(name="xb", bufs=1) as xbpool,
            tc.tile_pool(name="xe", bufs=1) as xepool,
            tc.tile_pool(name="ob", bufs=4) as opool,
            tc.tile_pool(name="ps", bufs=8, space="PSUM") as pspool,
        ):
            lw_sb = cpool.tile([128, KW, 122], bf16)
            nc.sync.dma_start(out=lw_sb[:, :, :], in_=lwT[:, :, :])
            gbt = cpool.tile([128, 2, NG], f32)
            nc.sync.dma_start(
                out=gbt[:, :, :],
                in_=gb[:, :].rearrange("a b -> (a b)").unsqueeze(0)
                .broadcast_to([128, 2 * NG]),
            )
            # Ring buffers sized a multiple of NBLK so each slot always
            # serves the same block index p: block 0's slots keep partitions
            # 0:3 (top pad rows) zero, block 4's keep 27:30 (bottom pad
            # rows) zero; DMA only ever writes the interior.
            NRING = 2 * NBLK
            xbufs = [xbpool.tile([128, WPAD], f32, name=f"xblk{j}")
                     for j in range(NRING)]
            xebufs = [xepool.tile([128, WPAD], bf16, name=f"xe{j}")
                      for j in range(NRING)]
            for j in range(NRING):
                nc.vector.memset(xbufs[j][:, :], 0.0)

            NT = IMGS_PER_CORE * NBLK
            LA = 8  # exp/load lookahead (software pipeline depth)

            def emit_load(k):
                # DMA pseudo-instructions occupy the issuing engine for the
                # whole transfer, so keep them off the busy ACT engine:
                # loads and stores alternate sync/gpsimd with opposite
                # parity so each ring gets a balanced mix.
                g, p = divmod(k, NBLK)
                r0, K = R0S[p], BKS[p]
                ldq = nc.sync if k % 2 == 0 else nc.gpsimd
                xblk = xbufs[k % NRING]
                # rows: padded row r holds phys row r0 + r - 3
                if p == 0:
                    ldq.dma_start(out=xblk[3:128, PAD:PAD + W],
                                  in_=x[g, 0:125, :])
                else:
                    lo = r0 - 3
                    ldq.dma_start(
                        out=xblk[0:K - (3 if p == NBLK - 1 else 0),
                                 PAD:PAD + W],
                        in_=x[g, lo:min(H, lo + K), :])

            def emit_exp(k):
                g, p = divmod(k, NBLK)
                K = BKS[p]
                nc.scalar.activation(
                    out=xebufs[k % NRING][0:K, :],
                    in_=xbufs[k % NRING][0:K, :], func=EXPF,
                    bias=gbt[0:K, 0, k:k + 1], scale=BETA,
                )

            for k in range(LA):
                emit_load(k)
                emit_exp(k)
            for k in range(NT):
                g, p = divmod(k, NBLK)
                r0, K, M = R0S[p], BKS[p], BMS[p]
                xe = xebufs[k % NRING]
                ps = pspool.tile([122, W], f32, tag="ps", name=f"ps{k}")
                for b in range(KW):
                    nc.tensor.matmul(
                        out=ps[0:M, :],
                        lhsT=lw_sb[0:K, b, 0:M],
                        rhs=xe[0:K, b:b + W],
                        start=(b == 0), stop=(b == KW - 1),
                    )
                ob = opool.tile([122, W], f32, tag="ob", name=f"ob{k}")
                # ln_k first (frees the psum bank, critical path), THEN the
                # lookahead exp (slack work) so ACT never gates the PE.
                nc.scalar.activation(out=ob[0:M, :], in_=ps[0:M, :],
                                     func=LNF,
                                     scale=float(np.exp(-CSHIFT)))
                if k + LA < NT:
                    emit_load(k + LA)
                    emit_exp(k + LA)
                nc.vector.tensor_scalar(
                    out=ob[0:M, :], in0=ob[0:M, :],
                    scalar1=1.0 / BETA,
                    scalar2=gbt[0:M, 1, k:k + 1],
                    op0=MUL, op1=ADD,
                )
                stq = nc.gpsimd if k % 2 == 0 else nc.sync
                stq.dma_start(out=out[g, r0:r0 + M, :], in_=ob[0:M, :])
    nc.finalize()
    return nc


def _expconv_host_inputs(xs_core, weight):
    """Per-core extra inputs: banded exp-weights + per-(image,block) shifts.

    gb row 0 feeds the Exp bias (S - beta*G_p); row 1 the final additive
    shift (G_p + wmax - (S+T)/beta).
    """
    import ml_dtypes

    w = np.asarray(weight, np.float64)
    wmax = float(w.max())
    lwT = np.zeros((128, KW, 122), np.float32)
    for a in range(KH):
        for b in range(KW):
            ev = float(np.exp(BETA * (w[a, b] - wmax) + TSHIFT))
            for i in range(122):
                r = i + a
                if r < 128:
                    lwT[r, b, i] = ev
    lwT = lwT.astype(ml_dtypes.bfloat16)
    G = np.zeros((IMGS_PER_CORE, NBLK), np.float32)
    for p, r0 in enumerate(R0S):
        lo = max(0, r0 - 3)
        hi = min(H, r0 - 3 + BKS[p])
        blk = xs_core[:, lo:hi, :].reshape(IMGS_PER_CORE, -1)
        G[:, p] = np.maximum(blk.max(axis=1), 0.0)
    gb = np.stack([
        SSHIFT - BETA * G.ravel(),
        G.ravel() + wmax + (CSHIFT - SSHIFT - TSHIFT) / BETA - OFFSET,
    ]).astype(np.float32)
    return lwT, gb


_NC_CACHE = {}


def _get_nc(weight, variant=None):
    variant = variant or VARIANT
    if variant == "w3":
        key = ("w3", np.asarray(weight, np.float32).tobytes())
    else:
        key = (variant,)
    if key not in _NC_CACHE:
        if variant == "w3":
            _NC_CACHE[key] = _build_w3(weight)
        elif variant == "f16p":
            _NC_CACHE[key] = _build_f16p()
        elif variant == "expconv":
            _NC_CACHE[key] = _build_expconv()
        else:
            raise ValueError(f"unknown variant {variant}")
    return _NC_CACHE[key]


def _run(x, weight, trace=False, variant=None, trace_kwargs=None):
    x = np.ascontiguousarray(x, dtype=np.float32)
    weight = np.ascontiguousarray(weight, dtype=np.float32)
    B, C, Hx, Wx = x.shape
    xs = x.reshape(B * C, Hx, Wx)
    per = (B * C) // N_CORES
    variant_r = variant or VARIANT
    if variant_r == "expconv":
        in_maps = []
        for i in range(N_CORES):
            xc = np.ascontiguousarray(xs[i * per:(i + 1) * per])
            lwT, gb = _expconv_host_inputs(xc, weight)
            in_maps.append({"x": xc, "lwT": lwT, "gb": gb})
    else:
        in_maps = [
            {"x": np.ascontiguousarray(xs[i * per:(i + 1) * per]),
             "weight": weight}
            for i in range(N_CORES)
        ]
    nc = _get_nc(weight, variant)
    res = run_bass_kernel_spmd(
        nc, in_maps, list(range(N_CORES)),
        trace=trace, trace_cores=[0] if trace else None,
        **(trace_kwargs or {}),
    )
    outs = np.concatenate([res.results[i]["out"] for i in range(N_CORES)], axis=0)
    return outs.reshape(B, C, Hx, Wx), res


def kernel(x, weight):
    out, _ = _run(x, weight)
    return out



# revision 25
# speedup vs baseline: 1.0250x; 1.0250x over previous
"""Morphological dilation (7x7 additive SE, zero 'same' padding) on 8 trn2 cores.

out[b,c,i,j] = max_{a,t} ( xpad[b,c,i+a,j+t] + w[a,t] ),  x: (8,8,512,512) f32.

Sharding: pure data parallel - 64 images (B*C) split 8 per core; the 7x7
weight is replicated (baked into the program as immediates). No cross-core
communication.

Default variant ("w3"): hand-authored custom DVE uop programs compute a
fused 3-tap sliding max-plus in ONE 1x Vector instruction:

    W3ACC:  out[s] = max(acc[s], x[s]+C2, x[s-1]+C1, x[s-2]+C0)

Per 2-image unit and vertical tap a (slab sh_a = row-shifted padded images,
[128, 8, 518] fp16; partition = row within 128-row strip, free = strip x
padded row):
  - horizontal taps b=0..2 and b=3..5: one W3ACC/W3INIT each (weights as
    instruction immediates via the delta trick C0-C2, C1-C2, C2),
  - tap b=6: ACT add (bias immediate) + one fp16 2x tensor_tensor max.
That is 22 DVE instructions per unit instead of ~66 for the tap-per-
instruction schedule: 49 taps cost 14x(4144+58)cyc + 8x(58+4096/2)cyc
~ 82us per unit on DVE vs ~136us before.

Accumulators are flat [128, 4152] fp16 tiles; each W3 group's output view
is offset by 6-b so its valid window lands at phys col r*518 + 8 + i for
every b. Stream positions whose 3-window crosses a row boundary write
garbage into cols {6-b, 7-b} < 8 of each row -- outside the extracted
region [8, 520) -- so no masking or subdim handling is needed. The first
two outputs of the whole stream are stale-pipeline garbage in the same
discarded columns.
"""

import os
import sys

for p in ("/root/.axon_site", "/root/.axon_site/_ro/trn_rl_repo",
          "/root/.axon_site/_ro/pypackages", "/opt/trn_rl_repo"):
    if os.path.isdir(p) and p not in sys.path:
        sys.path.append(p)

from dataclasses import dataclass

import numpy as np

import concourse.bass as bass  # noqa: F401
import concourse.bacc as bacc
import concourse.mybir as mybir
import concourse.dve_ops as dve_ops
from concourse.bass_utils import run_bass_kernel_spmd
from concourse.dve_spec import Spec, Src0, Src1, C0 as _C0, C1 as _C1, maxx
from concourse.dve_uop import (
    AluInp, AluOp, DelayInp, DveOpSpec, InpSel, OutPath, OutSel, Trigger,
    UopConfig, UopDpConfig, ENABLE,
)
from concourse.tile import TileContext

KH = KW = 7
PAD = 3
H = W = 512
N_CORES = 8
IMGS_PER_CORE = 8  # 8*8 = 64 images total
WPAD = W + 2 * PAD  # 518
S = H // 128  # 4 strips of 128 rows per image
SR = 8  # slab rows per 2-image unit (2 images x 4 strips)
FLAT = SR * WPAD  # 4144
ACCW = 4152  # acc tile width (>= 6 + FLAT, even)

f32 = mybir.dt.float32
f16 = mybir.dt.float16
ADD = mybir.AluOpType.add
MAX = mybir.AluOpType.max
IDENT = mybir.ActivationFunctionType.Identity

VARIANT = os.environ.get("BASS_DILATE_VARIANT", "w3")
N_ACT_EXTRA = int(os.environ.get("BASS_DILATE_ACT_EXTRA", "13"))

# ---- expconv variant constants -------------------------------------------
# Dilation via log-sum-exp: out = G + wmax + (1/beta)*( ln(conv2d(
# e^{beta(x-G)+S}, e^{beta(w-wmax)+T})) - S - T ), computed as banded
# matmuls on the PE. Row-blocks of 128 padded rows at stride 122 (out-rows
# per block M=122); per-(image,block) shift G = max(block max, 0).
# S and T re-center each bf16 factor: the dominant tap of a window can sit
# e^{-beta*D} below its factor's peak with D_x + D_w = G + wmax - out
# (measured <= 6.43 on the real data; D_w <= w spread 5.19). The HW ACT Ln
# table is only valid for inputs in ~[1.2e-38, 3e19], so Ln gets a pre-scale
# e^{-C} to recenter E (its ~126-nat span fits the ~131-nat valid window).
# Validated in numpy (bf16 factors, f32 accum): beta=19 -> worst rel ~0.011.
BETA = float(os.environ.get("BASS_DILATE_BETA", "12.5"))
SSHIFT = float(os.environ.get("BASS_DILATE_S", "42.0"))
TSHIFT = float(os.environ.get("BASS_DILATE_T", "42.0"))
CSHIFT = float(os.environ.get("BASS_DILATE_C", "45.5"))
# The log-sum-exp bias is one-sided (overestimates); subtracting its midpoint
# halves the worst-case error. Measured on the real data at beta=12.5.
OFFSET = float(os.environ.get("BASS_DILATE_OFF", "0.0655"))
R0S = (0, 122, 244, 366, 488)  # padded-row block starts
BKS = (128, 128, 128, 128, 30)  # contraction size (valid padded rows)
BMS = (122, 122, 122, 122, 24)  # out rows per block
NBLK = len(R0S)
bf16 = mybir.dt.bfloat16


# --------------------------------------------------------------------------
# Custom DVE ops: fused 3-tap sliding max-plus (hand-authored v3 uops).
#
# 8-block datapath, 1x mode:
#   b0: a2 = ADD(ch0=x, ch4=c2)           ; ch5 <- CURR(b0)  = a2[s-1]
#   b1: BYPASS(ch5) -> flop a2[s-1]       ; ch0 <- CURR(b1)  = a2[s-2]
#                                         ; ch4 <- PREV_ALU  = a2[s]
#   b2: a1 = ADD(PREV_ALU, ch3=d1)
#   b3: a0 = ADD(ch0, ch2=d0)             ; ch5 <- PREV_ALU  = a1
#   b4: m1 = MAX(PREV_ALU=a0, ch5=a1)
#   b5: m2 = MAX(PREV_ALU, ch4=a2[s])
#   b6: r  = MAX(PREV_ALU, ch1=acc)       [W3INIT: BYPASS]
#   b7: BYPASS -> WR0_LO
# --------------------------------------------------------------------------


def _w3_uop(with_acc: bool) -> UopConfig:
    u = UopConfig()
    u.enable_input(InpSel.SRC_0, 1)      # chain0 = x
    if with_acc:
        u.enable_input(InpSel.SRC_1, 2)  # chain1 = acc
    u.enable_input(InpSel.CONST_0, 3)    # chain2 = d0 = c0-c2
    u.enable_input(InpSel.CONST_1, 4)    # chain3 = d1 = c1-c2
    u.enable_input(InpSel.CONST_2, 5)    # chain4 = c2
    u.require_inp0 = ENABLE
    u.require_inp1 = ENABLE if with_acc else 0
    u.trigger = (Trigger.SRC_TENSOR_DONE, Trigger.NONE, Trigger.NONE)
    u.next_uop = (0, 0, 0)
    u.repeat_count = 0
    u.out[OutPath.WR0_LO] = OutSel.ALU_OUT
    u.out_enable[OutPath.WR0_LO] = ENABLE

    dp = [UopDpConfig() for _ in range(8)]
    dp[0].enable_alu(AluOp.ADD, AluInp.PREV_DELAY_0, AluInp.PREV_DELAY_4)
    dp[0].pass_through_delay(1, 2, 3)
    dp[0].enable_delay_from_src(DelayInp.CURR_ALU_OUT, 5)
    dp[1].enable_alu(AluOp.BYPASS, AluInp.PREV_DELAY_5, AluInp.PREV_DELAY_5)
    dp[1].pass_through_delay(1, 2, 3)
    dp[1].enable_delay_from_src(DelayInp.CURR_ALU_OUT, 0)
    dp[1].enable_delay_from_src(DelayInp.PREV_ALU_OUT, 4)
    dp[2].enable_alu(AluOp.ADD, AluInp.PREV_ALU_OUT, AluInp.PREV_DELAY_3)
    dp[2].pass_through_delay(0, 1, 2, 4)
    dp[3].enable_alu(AluOp.ADD, AluInp.PREV_DELAY_0, AluInp.PREV_DELAY_2)
    dp[3].pass_through_delay(1, 4)
    dp[3].enable_delay_from_src(DelayInp.PREV_ALU_OUT, 5)
    dp[4].enable_alu(AluOp.MAX, AluInp.PREV_ALU_OUT, AluInp.PREV_DELAY_5)
    dp[4].pass_through_delay(1, 4)
    dp[5].enable_alu(AluOp.MAX, AluInp.PREV_ALU_OUT, AluInp.PREV_DELAY_4)
    dp[5].pass_through_delay(1)
    if with_acc:
        dp[6].enable_alu(AluOp.MAX, AluInp.PREV_ALU_OUT, AluInp.PREV_DELAY_1)
    else:
        dp[6].pass_through_alu()
    dp[7].pass_through_alu()
    u.datapath_config = dp
    return u


def _w3_ref_core(in0, s0, s1, imm2):
    p = in0.shape[0]
    x = np.asarray(in0, np.float32).reshape(p, -1)
    a2 = x + float(imm2)
    ninf = np.float32(-np.inf)
    s0 = float(np.asarray(s0).flat[0]) if not isinstance(s0, float) else s0
    s1 = float(np.asarray(s1).flat[0]) if not isinstance(s1, float) else s1
    a1 = np.concatenate([np.full((p, 1), ninf, np.float32), a2[:, :-1] + s1], 1)
    a0 = np.concatenate([np.full((p, 2), ninf, np.float32), a2[:, :-2] + s0], 1)
    return np.maximum(np.maximum(a0, a1), a2)


def _w3acc_ref(in0, in1, s0, s1, imm2):
    r = _w3_ref_core(in0, s0, s1, imm2)
    r = np.maximum(r, np.asarray(in1, np.float32).reshape(r.shape))
    return r.reshape(np.asarray(in0).shape)


def _w3init_ref(in0, in1, s0, s1, imm2):
    return _w3_ref_core(in0, s0, s1, imm2).reshape(np.asarray(in0).shape)


@dataclass(frozen=True)
class _HandOp:
    """Duck-typed DveOp with a hand-authored uop program."""

    name: str
    spec: Spec
    subdim: bool
    uopspec: DveOpSpec

    def compile(self, ver):
        assert ver == "v3", f"W3 ops are v3-only (got {ver})"
        return self.uopspec


_W3_OPS = None


def _register_w3():
    global _W3_OPS
    if _W3_OPS is not None:
        return _W3_OPS
    by_name = {op.name: op for op in dve_ops.OPS}
    if "W3ACC_ANT" in by_name:  # registered by a sibling module/import
        _W3_OPS = (by_name["W3ACC_ANT"], by_name["W3INIT_ANT"])
        return _W3_OPS
    base = dve_ops._CUSTOM_DVE_ROW_BASE
    row_acc = base + len(dve_ops.OPS)
    row_init = row_acc + 1
    assert row_init < 0x20
    w3acc = _HandOp(
        name="W3ACC_ANT",
        spec=Spec(body=maxx(Src0 + _C0, Src1), reference=_w3acc_ref),
        subdim=False,
        uopspec=DveOpSpec(name="W3ACC_ANT", opcode=row_acc,
                          uops=[_w3_uop(True)], rd1_en=True),
    )
    w3init = _HandOp(
        name="W3INIT_ANT",
        spec=Spec(body=Src0 + _C0 + _C1, reference=_w3init_ref),
        subdim=False,
        uopspec=DveOpSpec(name="W3INIT_ANT", opcode=row_init,
                          uops=[_w3_uop(False)], rd1_en=False),
    )
    for op in (w3acc, w3init):
        op.uopspec.validate("v3")
        dve_ops.OPS.append(op)
        dve_ops._SUB_OPCODE_FOR_NAME[op.name] = op.uopspec.opcode
        dve_ops.CUSTOM_DVE_SPECS[op.name] = op.spec
    _W3_OPS = (w3acc, w3init)
    return _W3_OPS


# --------------------------------------------------------------------------
# Kernel builders
# --------------------------------------------------------------------------


def _emit_pad_fill(nc, xpad, zt, g):
    """Zero xpad[g] (interior gets overwritten by the image afterwards)."""
    for r0 in range(0, WPAD, 128):
        r1 = min(WPAD, r0 + 128)
        nc.sync.dma_start(out=xpad[g, r0:r1, :], in_=zt[0:r1 - r0, :])


def _emit_border_fill(nc, xpad, zt, g):
    """Zero only the 3-wide borders of xpad[g]; the interior fill can then
    run concurrently instead of waiting for a full-frame zero pass."""
    nc.sync.dma_start(out=xpad[g, 0:PAD, :], in_=zt[0:PAD, :])
    nc.sync.dma_start(out=xpad[g, PAD + H:WPAD, :], in_=zt[0:PAD, :])
    # column borders, 128 rows at a time
    for r0 in range(0, H, 128):
        nc.sync.dma_start(out=xpad[g, PAD + r0:PAD + r0 + 128, 0:PAD],
                          in_=zt[0:128, 0:PAD])
        nc.sync.dma_start(out=xpad[g, PAD + r0:PAD + r0 + 128, PAD + W:WPAD],
                          in_=zt[0:128, 0:PAD])


def _build_w3(weight):
    w3acc_op, w3init_op = _register_w3()
    w = np.asarray(weight, np.float64)
    nc = bacc.Bacc("TRN2")
    x = nc.dram_tensor("x", (IMGS_PER_CORE, H, W), f32, kind="ExternalInput")
    wt = nc.dram_tensor("weight", (KH, KW), f32, kind="ExternalInput")
    out = nc.dram_tensor("out", (IMGS_PER_CORE, H, W), f32, kind="ExternalOutput")
    NP = IMGS_PER_CORE // 2
    NCH = 2

    with TileContext(nc) as tc:
        with (
            tc.tile_pool(name="const", bufs=1) as cpool,
            tc.tile_pool(name="dram", bufs=1, space="DRAM") as dpool,
            tc.tile_pool(name="sh", bufs=10) as shpool,
            tc.tile_pool(name="tmp", bufs=6) as tpool,
            tc.tile_pool(name="acc", bufs=3) as apool,
        ):
            # W3 weights ride as immediates; the broadcast tile feeds the
            # ACT-path (tap b=6) per-partition biases.
            w_sb = cpool.tile([128, KH * KW], f32)
            nc.sync.dma_start(
                out=w_sb[:, :],
                in_=wt[:, :].rearrange("a b -> (a b)").unsqueeze(0)
                .broadcast_to([128, KH * KW]),
            )
            zt = cpool.tile([128, WPAD], f16)
            nc.vector.memset(zt[:, :], 0.0)

            xpad = dpool.tile([IMGS_PER_CORE, WPAD, WPAD], f16)

            def fill(g):
                _emit_pad_fill(nc, xpad, zt, g)
                nc.gpsimd.dma_start(
                    out=xpad[g, PAD:PAD + H, PAD:PAD + W], in_=x[g, :, :]
                )

            for g in range(min(4, IMGS_PER_CORE)):
                fill(g)

            for u in range(NP):
                g0 = 2 * u
                accs = [
                    apool.tile([128, ACCW], f16, tag=f"acc{c}", name=f"acc{c}_{u}")
                    for c in range(NCH)
                ]
                for c in range(NCH):
                    nc.vector.memset(accs[c][:, 0:6], 0.0)
                acc_used = [False] * NCH

                def acc_valid(c):
                    return (accs[c][:, 8:8 + FLAT]
                            .rearrange("p (s w) -> p s w", w=WPAD)[:, :, 0:W])

                op_idx = 0
                for a in range(KH):
                    sh = shpool.tile([128, SR, WPAD], f16, tag="sh",
                                     name=f"sh_{u}_{a}")
                    for gg in range(2):
                        nc.sync.dma_start(
                            out=sh[:, 4 * gg:4 * gg + 4, :],
                            in_=xpad[g0 + gg, a:a + H, :]
                            .rearrange("(s p) w -> p s w", p=128),
                        )
                    # ACT's tap b=6 add is emitted first so the Scalar engine
                    # starts on the slab immediately; its max joins later.
                    tmp = tpool.tile([128, SR, W], f16, tag="tmp",
                                     name=f"tmp_{u}_{a}")
                    nc.scalar.activation(
                        out=tmp[:, :, :], in_=sh[:, :, 6:6 + W], func=IDENT,
                        bias=w_sb[:, a * KW + 6:a * KW + 7], scale=1.0,
                    )
                    for b in (0, 3):
                        c = op_idx % NCH
                        op_idx += 1
                        o = 6 - b
                        kwargs = dict(
                            out=accs[c][:, o:o + FLAT],
                            in0=sh[:, :, :],
                            s0=float(w[a, b] - w[a, b + 2]),
                            s1=float(w[a, b + 1] - w[a, b + 2]),
                            imm2=float(w[a, b + 2]),
                        )
                        if acc_used[c]:
                            nc.vector._custom_dve(
                                w3acc_op, in1=accs[c][:, o:o + FLAT], **kwargs)
                        else:
                            nc.vector._custom_dve(w3init_op, **kwargs)
                            acc_used[c] = True
                    c = op_idx % NCH
                    op_idx += 1
                    nc.vector.tensor_tensor(
                        out=acc_valid(c), in0=acc_valid(c), in1=tmp[:, :, :],
                        op=MAX,
                    )
                # combine + store per image so the first output DMA overlaps
                # the second image's combine
                for gg in range(2):
                    r0, r1 = 4 * gg, 4 * gg + 4
                    for c in range(1, NCH):
                        nc.vector.tensor_tensor(
                            out=acc_valid(0)[:, r0:r1, :],
                            in0=acc_valid(0)[:, r0:r1, :],
                            in1=acc_valid(c)[:, r0:r1, :],
                            op=MAX,
                        )
                    nc.gpsimd.dma_start(
                        out=out[g0 + gg].rearrange("(s p) w -> p s w", p=128),
                        in_=acc_valid(0)[:, r0:r1, :],
                    )
                for g in (2 * u + 4, 2 * u + 5):
                    if g < IMGS_PER_CORE:
                        fill(g)
    nc.finalize()
    return nc


def _build_f16p():
    """Fallback: tap-per-instruction schedule (DVE tt/ts + ACT adds)."""
    nc = bacc.Bacc("TRN2")
    x = nc.dram_tensor("x", (IMGS_PER_CORE, H, W), f32, kind="ExternalInput")
    wt = nc.dram_tensor("weight", (KH, KW), f32, kind="ExternalInput")
    out = nc.dram_tensor("out", (IMGS_PER_CORE, H, W), f32, kind="ExternalOutput")
    NCH = int(os.environ.get("BASS_DILATE_NCH", "4"))
    NP = IMGS_PER_CORE // 2

    act_taps = {(a, t) for a in range(KH) for t in range(KW) if t % 2 == 1}
    even_taps = [(a, t) for a in range(KH) for t in range(KW) if t % 2 == 0]
    step = max(1, len(even_taps) // max(1, N_ACT_EXTRA))
    for i in range(0, min(N_ACT_EXTRA, len(even_taps))):
        act_taps.add(even_taps[(i * step) % len(even_taps)])

    with TileContext(nc) as tc:
        with (
            tc.tile_pool(name="const", bufs=1) as cpool,
            tc.tile_pool(name="dram", bufs=1, space="DRAM") as dpool,
            tc.tile_pool(name="sh", bufs=10) as shpool,
            tc.tile_pool(name="tmp", bufs=5) as tpool,
            tc.tile_pool(name="acc", bufs=3) as apool,
        ):
            w_sb = cpool.tile([128, KH * KW], f32)
            nc.sync.dma_start(
                out=w_sb[:, :],
                in_=wt[:, :].rearrange("a b -> (a b)").unsqueeze(0)
                .broadcast_to([128, KH * KW]),
            )
            zt = cpool.tile([128, WPAD], f16)
            nc.vector.memset(zt[:, :], 0.0)

            xpad = dpool.tile([IMGS_PER_CORE, WPAD, WPAD], f16)

            def fill(g):
                _emit_pad_fill(nc, xpad, zt, g)
                nc.gpsimd.dma_start(
                    out=xpad[g, PAD:PAD + H, PAD:PAD + W], in_=x[g, :, :]
                )

            for g in range(min(4, IMGS_PER_CORE)):
                fill(g)
            for u in range(NP):
                g0 = 2 * u
                accs = [
                    apool.tile([128, 2, S, W], f16, tag=f"acc{c}", name=f"acc{c}_{u}")
                    for c in range(NCH)
                ]
                acc_used = [False] * NCH
                for a in range(KH):
                    sh = shpool.tile([128, 2, S, WPAD], f16, tag="sh",
                                     name=f"sh_{u}_{a}")
                    for gg in range(2):
                        nc.sync.dma_start(
                            out=sh[:, gg, :, :],
                            in_=xpad[g0 + gg, a:a + H, :]
                            .rearrange("(s p) w -> p s w", p=128),
                        )
                    for t in range(KW):
                        k = a * KW + t
                        c = k % NCH
                        in0 = sh[:, :, :, t:t + W]
                        if not acc_used[c]:
                            dst = accs[c][:, :, :, :]
                        else:
                            tmp = tpool.tile([128, 2, S, W], f16, tag="tmp",
                                             name=f"tmp_{u}_{k}")
                            dst = tmp[:, :, :, :]
                        if (a, t) in act_taps:
                            nc.scalar.activation(
                                out=dst, in_=in0, func=IDENT,
                                bias=w_sb[:, k:k + 1], scale=1.0,
                            )
                        else:
                            nc.vector.tensor_scalar(
                                out=dst, in0=in0,
                                scalar1=w_sb[:, k:k + 1], scalar2=None, op0=ADD,
                            )
                        if acc_used[c]:
                            nc.vector.tensor_tensor(
                                out=accs[c][:, :, :, :], in0=accs[c][:, :, :, :],
                                in1=dst, op=MAX,
                            )
                        acc_used[c] = True
                for c in range(1, NCH):
                    nc.vector.tensor_tensor(
                        out=accs[0][:, :, :, :], in0=accs[0][:, :, :, :],
                        in1=accs[c][:, :, :, :], op=MAX,
                    )
                for gg in range(2):
                    nc.gpsimd.dma_start(
                        out=out[g0 + gg].rearrange("(s p) w -> p s w", p=128),
                        in_=accs[0][:, gg, :, :],
                    )
                for g in (2 * u + 4, 2 * u + 5):
                    if g < IMGS_PER_CORE:
                        fill(g)
    nc.finalize()
    return nc


def _patch_act_tables():
    """Force the Exp+Ln co-resident act-func set (natural_log_exp_and_others)
    so alternating Exp/Ln activations don't reload tables (1.3us per reload,
    ~92us/run). Preserves set order/count so act_func_set_id indices stay
    valid; just strips exp/ln from every other set."""
    import concourse.bacc as _bacc
    import concourse.hw_specs as _hw

    if getattr(_bacc, "_ant_act_patch", False):
        return
    real = _hw.get_activation_tables

    def patched(arch):
        tables = dict(real(arch))
        both = {
            n for n, fs in tables.items()
            if mybir.ActivationFunctionType.Exp in fs
            and mybir.ActivationFunctionType.Ln in fs
        }
        if not both:
            return tables
        keep = next(iter(sorted(both)))
        out = {}
        for n, fs in tables.items():
            if n == keep:
                out[n] = fs
            else:
                out[n] = fs - {mybir.ActivationFunctionType.Exp,
                               mybir.ActivationFunctionType.Ln}
        return out

    _bacc.get_activation_tables = patched
    _bacc._ant_act_patch = True


def _build_expconv():
    """Dilation as exp-domain 7x7 convolution on the tensor engine.

    out[i,j] = max_{a,b} xpad[i+a, j+b] + w[a,b]
            ~= G_p + (1/beta) ln( sum_{a,b} e^{beta w[a,b]} e^{beta(xpad[i+a,j+b]-G_p)} )

    The inner sum is a linear 7x7 conv of Xe = e^{beta(xpad-G_p)} with kernel
    e^{beta w}: for each horizontal tap b, one banded matmul contracting over
    the 128 padded rows of block p (lhsT[r', i'] = e^{beta w[r'-i', b]} band),
    rhs = Xe free-shifted by b, accumulating all 7 taps in PSUM. ACT does
    exp and ln, Pool does the final (1/beta)*ln + G_p, DVE is idle.
    lwT (banded exp-weights) and gb (per-block shifts) are host-computed
    inputs, so the program is weight/data independent.
    """
    _patch_act_tables()
    nc = bacc.Bacc("TRN2")
    x = nc.dram_tensor("x", (IMGS_PER_CORE, H, W), f32, kind="ExternalInput")
    lwT = nc.dram_tensor("lwT", (128, KW, 122), bf16, kind="ExternalInput")
    gb = nc.dram_tensor("gb", (2, IMGS_PER_CORE * NBLK), f32,
                        kind="ExternalInput")
    out = nc.dram_tensor("out", (IMGS_PER_CORE, H, W), f32,
                         kind="ExternalOutput")
    NG = IMGS_PER_CORE * NBLK
    EXPF = mybir.ActivationFunctionType.Exp
    LNF = mybir.ActivationFunctionType.Ln
    MUL = mybir.AluOpType.mult

    with TileContext(nc) as tc:
        with (
            tc.tile_pool(name="const", bufs=1) as cpool,
            tc.tile_pool(name="xb", bufs=1) as xbpool,
            tc.tile_pool(name="xe", bufs=1) as xepool,
            tc.tile_pool(name="ob", bufs=4) as opool,
            tc.tile_pool(name="ps", bufs=8, space="PSUM") as pspool,
        ):
            lw_sb = cpool.tile([128, KW, 122], bf16)
            nc.sync.dma_start(out=lw_sb[:, :, :], in_=lwT[:, :, :])
            gbt = cpool.tile([128, 2, NG], f32)
            nc.sync.dma_start(
                out=gbt[:, :, :],
                in_=gb[:, :].rearrange("a b -> (a b)").unsqueeze(0)
                .broadcast_to([128, 2 * NG]),
            )
            # Ring buffers sized a multiple of NBLK so each slot always
            # serves the same block index p: block 0's slots keep partitions
            # 0:3 (top pad rows) zero, block 4's keep 27:30 (bottom pad
            # rows) zero; DMA only ever writes the interior.
            NRING = 2 * NBLK
            xbufs = [xbpool.tile([128, WPAD], f32, name=f"xblk{j}")
                     for j in range(NRING)]
            xebufs = [xepool.tile([128, WPAD], bf16, name=f"xe{j}")
                      for j in range(NRING)]
            # Zero only the pad regions (DMA never writes them): pad cols
            # for every slot, top pad rows for p==0 slots, bottom pad rows
            # for p==NBLK-1 slots. Small memsets so prologue loads aren't
            # gated behind full-tile zeroing.
            for j in range(NRING):
                p = j % NBLK
                nc.vector.memset(xbufs[j][:, 0:PAD], 0.0)
                nc.vector.memset(xbufs[j][:, PAD + W:WPAD], 0.0)
                if p == 0:
                    nc.vector.memset(xbufs[j][0:PAD, :], 0.0)
                if p == NBLK - 1:
                    # engines need 32-aligned partition base; zero [0:32]
                    # before any DMA (the load later rewrites [0:27])
                    nc.vector.memset(xbufs[j][0:32, :], 0.0)

            NT = IMGS_PER_CORE * NBLK
            LA = 8  # exp/load lookahead (software pipeline depth)

            def emit_load(k):
                # DMA pseudo-instructions occupy the issuing engine for the
                # whole transfer, so keep them off the busy ACT engine:
                # loads and stores alternate sync/gpsimd with opposite
                # parity so each ring gets a balanced mix.
                g, p = divmod(k, NBLK)
                r0, K = R0S[p], BKS[p]
                ldq = nc.sync
                xblk = xbufs[k % NRING]
                # rows: padded row r holds phys row r0 + r - 3
                if p == 0:
                    ldq.dma_start(out=xblk[3:128, PAD:PAD + W],
                                  in_=x[g, 0:125, :])
                else:
                    lo = r0 - 3
                    ldq.dma_start(
                        out=xblk[0:K - (3 if p == NBLK - 1 else 0),
                                 PAD:PAD + W],
                        in_=x[g, lo:min(H, lo + K), :])

            def emit_exp(k):
                g, p = divmod(k, NBLK)
                K = BKS[p]
                nc.scalar.activation(
                    out=xebufs[k % NRING][0:K, :],
                    in_=xbufs[k % NRING][0:K, :], func=EXPF,
                    bias=gbt[0:K, 0, k:k + 1], scale=BETA,
                )

            for k in range(LA):
                emit_load(k)
                emit_exp(k)
            for k in range(NT):
                g, p = divmod(k, NBLK)
                r0, K, M = R0S[p], BKS[p], BMS[p]
                xe = xebufs[k % NRING]
                ps = pspool.tile([122, W], f32, tag="ps", name=f"ps{k}")
                for b in range(KW):
                    nc.tensor.matmul(
                        out=ps[0:M, :],
                        lhsT=lw_sb[0:K, b, 0:M],
                        rhs=xe[0:K, b:b + W],
                        start=(b == 0), stop=(b == KW - 1),
                    )
                ob = opool.tile([122, W], f32, tag="ob", name=f"ob{k}")
                # ln_k first (frees the psum bank, critical path), THEN the
                # lookahead exp (slack work) so ACT never gates the PE.
                nc.scalar.activation(out=ob[0:M, :], in_=ps[0:M, :],
                                     func=LNF,
                                     scale=float(np.exp(-CSHIFT)))
                if k + LA < NT:
                    emit_load(k + LA)
                    emit_exp(k + LA)
                nc.vector.tensor_scalar(
                    out=ob[0:M, :], in0=ob[0:M, :],
                    scalar1=1.0 / BETA,
                    scalar2=gbt[0:M, 1, k:k + 1],
                    op0=MUL, op1=ADD,
                )
                nc.gpsimd.dma_start(out=out[g, r0:r0 + M, :], in_=ob[0:M, :])
    nc.finalize()
    return nc


def _expconv_host_inputs(xs_core, weight):
    """Per-core extra inputs: banded exp-weights + per-(image,block) shifts.

    gb row 0 feeds the Exp bias (S - beta*G_p); row 1 the final additive
    shift (G_p + wmax - (S+T)/beta).
    """
    import ml_dtypes

    w = np.asarray(weight, np.float64)
    wmax = float(w.max())
    lwT = np.zeros((128, KW, 122), np.float32)
    for a in range(KH):
        for b in range(KW):
            ev = float(np.exp(BETA * (w[a, b] - wmax) + TSHIFT))
            for i in range(122):
                r = i + a
                if r < 128:
                    lwT[r, b, i] = ev
    lwT = lwT.astype(ml_dtypes.bfloat16)
    G = np.zeros((IMGS_PER_CORE, NBLK), np.float32)
    for p, r0 in enumerate(R0S):
        lo = max(0, r0 - 3)
        hi = min(H, r0 - 3 + BKS[p])
        blk = xs_core[:, lo:hi, :].reshape(IMGS_PER_CORE, -1)
        G[:, p] = np.maximum(blk.max(axis=1), 0.0)
    gb = np.stack([
        SSHIFT - BETA * G.ravel(),
        G.ravel() + wmax + (CSHIFT - SSHIFT - TSHIFT) / BETA - OFFSET,
    ]).astype(np.float32)
    return lwT, gb


_NC_CACHE = {}


def _get_nc(weight, variant=None):
    variant = variant or VARIANT
    if variant == "w3":
        key = ("w3", np.asarray(weight, np.float32).tobytes())
    else:
        key = (variant,)
    if key not in _NC_CACHE:
        if variant == "w3":
            _NC_CACHE[key] = _build_w3(weight)
        elif variant == "f16p":
            _NC_CACHE[key] = _build_f16p()
        elif variant == "expconv":
            _NC_CACHE[key] = _build_expconv()
        else:
            raise ValueError(f"unknown variant {variant}")
    return _NC_CACHE[key]


def _run(x, weight, trace=False, variant=None, trace_kwargs=None):
    x = np.ascontiguousarray(x, dtype=np.float32)
    weight = np.ascontiguousarray(weight, dtype=np.float32)
    B, C, Hx, Wx = x.shape
    xs = x.reshape(B * C, Hx, Wx)
    per = (B * C) // N_CORES
    variant_r = variant or VARIANT
    if variant_r == "expconv":
        in_maps = []
        for i in range(N_CORES):
            xc = np.ascontiguousarray(xs[i * per:(i + 1) * per])
            lwT, gb = _expconv_host_inputs(xc, weight)
            in_maps.append({"x": xc, "lwT": lwT, "gb": gb})
    else:
        in_maps = [
            {"x": np.ascontiguousarray(xs[i * per:(i + 1) * per]),
             "weight": weight}
            for i in range(N_CORES)
        ]
    nc = _get_nc(weight, variant)
    res = run_bass_kernel_spmd(
        nc, in_maps, list(range(N_CORES)),
        trace=trace, trace_cores=[0] if trace else None,
        **(trace_kwargs or {}),
    )
    outs = np.concatenate([res.results[i]["out"] for i in range(N_CORES)], axis=0)
    return outs.reshape(B, C, Hx, Wx), res


def kernel(x, weight):
    out, _ = _run(x, weight)
    return out

